# revision 1
# baseline (speedup 1.0000x reference)
"""Trainium2 Bass kernel v6 for nn_BTokenizer (residual MLP tokenizer block).

Computes, for x [16384, 1024]:
    y = x @ Win.T + bin
    6x: y = y + gelu(LN(y) @ Wb[i].T + bb[i])
    out = y @ Wout.T + bout          -> [16384, 2048]

Data-parallel over tokens (2048/core). FEATURE-MAJOR resident state: y is
kept as y.T [feature(part), token(free)] in bf16; weights are the stationary
matmul operand (ldweights fully hidden, 512-col streams); no transposes.
LayerNorm mean-subtraction is folded into the weights on the host
(W' = W - outer(rowsum(W), ones)/D), so the normalize is a single multiply
by rstd. Variance comes from elementwise kt-partial reduction trees (Pool
for y, DVE for y^2) + a single ones-matmul partition reduction per chunk.
Bias+GELU fuse into the Act engine's PSUM evacuation (bias per-partition).

v6: the block body is software-pipelined over token HALVES (c01 / c23):
each half's stats, rstd and next-block normalize execute during the other
half's matmul window, so the PE never waits at block boundaries.
"""

import contextlib

import numpy as np

import concourse.bass as bass
import concourse.tile as tile
from concourse import bacc, mybir
from concourse import bass_utils

F32 = mybir.dt.float32
BF16 = mybir.dt.bfloat16
AF = mybir.ActivationFunctionType
OP = mybir.AluOpType

N_CORES = 8
N_TOKENS = 16384
T = N_TOKENS // N_CORES  # 2048 tokens per core
D = 1024                 # in/hidden width
NOUT = 2048
NB = 6                   # inner residual blocks
EPS = 1e-5
KT = D // 128            # 8 k-tiles (feature partitions)
FT = D // 128            # 8 fout-tiles for hidden layers
FT3 = NOUT // 128        # 16 fout-tiles for the output layer
NC4 = T // 512           # 4 psum-bank token chunks
NC2 = T // 1024          # 2 token halves


def build_nc(repeat: int = 1):
    nc = bacc.Bacc("TRN2", target_bir_lowering=False, debug=False, num_devices=1)

    xt = nc.dram_tensor("xt", [D, T], BF16, kind="ExternalInput").ap()
    wint = nc.dram_tensor("wint", [D, D], BF16, kind="ExternalInput").ap()
    wbt = nc.dram_tensor("wbt", [NB, D, D], BF16, kind="ExternalInput").ap()
    woutt = nc.dram_tensor("woutt", [D, NOUT], BF16, kind="ExternalInput").ap()
    bin_b = nc.dram_tensor("bin_b", [D], F32, kind="ExternalInput").ap()
    bb = nc.dram_tensor("bb", [NB, D], F32, kind="ExternalInput").ap()
    bout = nc.dram_tensor("bout", [NOUT], F32, kind="ExternalInput").ap()
    out_t = nc.dram_tensor("out_t", [NOUT, T], F32, kind="ExternalOutput").ap()

    with tile.TileContext(nc) as tc:
        with contextlib.ExitStack() as ctx:
            kernel_body(ctx, tc, xt, wint, wbt, woutt, bin_b, bb, bout, out_t,
                        repeat)
    nc.finalize()
    return nc


def kernel_body(ctx, tc, xt, wint, wbt, woutt, bin_b, bb, bout, out_t, repeat):
    nc = tc.nc

    singles = ctx.enter_context(tc.tile_pool(name="singles", bufs=1))
    ypool = ctx.enter_context(tc.tile_pool(name="ypool", bufs=1))
    tpool = ctx.enter_context(tc.tile_pool(name="tpool", bufs=1))
    wpool = ctx.enter_context(tc.tile_pool(name="wpool", bufs=2))
    gpool = ctx.enter_context(tc.tile_pool(name="gpool", bufs=4))
    ysqpool = ctx.enter_context(tc.tile_pool(name="ysqpool", bufs=2))
    statpool = ctx.enter_context(tc.tile_pool(name="statpool", bufs=2))
    rstdpool = ctx.enter_context(tc.tile_pool(name="rstdpool", bufs=2))
    ostpool = ctx.enter_context(tc.tile_pool(name="ostpool", bufs=2))
    psG = ctx.enter_context(tc.tile_pool(name="psG", bufs=3, space="PSUM"))
    psS = ctx.enter_context(tc.tile_pool(name="psS", bufs=1, space="PSUM"))

    # constants / biases (per-partition layouts)
    ones128 = singles.tile([128, 128], BF16)
    nc.vector.memset(ones128, 1.0)
    eps = singles.tile([128, 1], F32)
    nc.vector.memset(eps, EPS)
    bin_t = singles.tile([128, FT], F32)
    nc.sync.dma_start(bin_t, bin_b.rearrange("(f p) -> p f", p=128))
    bb_t = singles.tile([128, NB, FT], F32)
    nc.sync.dma_start(bb_t, bb.rearrange("i (f p) -> p i f", p=128))
    bout_t = singles.tile([128, FT3], F32)
    nc.sync.dma_start(bout_t, bout.rearrange("(f p) -> p f", p=128))

    # resident state: y.T and t = x.T / normalized y.T, feature-major bf16
    y_t = ypool.tile([128, KT, T], BF16)
    t_t = tpool.tile([128, KT, T], BF16)

    # ---- stats helpers (per token half: 2 chunks of 512) ----
    def make_stats(half):
        sa = [[ysqpool.tile([128, 512], BF16, name="sp", tag=f"sp{j}_{c}")
               for c in range(2)] for j in range(4)]
        qa = [[ysqpool.tile([128, 512], BF16, name="qp", tag=f"qp{j}_{c}")
               for c in range(2)] for j in range(4)]
        return half, sa, qa

    def stats_pair(stats, f):
        """Level-1 kt-partials for feature pair (f-1, f) on this half's two
        chunks: y on Pool, y^2 on DVE."""
        half, sa, qa = stats
        j = f // 2
        for ci in range(2):
            cs = bass.ts(2 * half + ci, 512)
            nc.gpsimd.tensor_tensor(sa[j][ci], y_t[:, f - 1, cs],
                                    y_t[:, f, cs], OP.add)
            q0 = ysqpool.tile([128, 512], BF16, name="ysq", tag="ysq0")
            nc.vector.tensor_tensor(q0, y_t[:, f - 1, cs], y_t[:, f - 1, cs],
                                    OP.mult)
            q1 = ysqpool.tile([128, 512], BF16, name="ysq", tag="ysq1")
            nc.vector.tensor_tensor(q1, y_t[:, f, cs], y_t[:, f, cs], OP.mult)
            nc.vector.tensor_tensor(qa[j][ci], q0, q1, OP.add)

    def stats_finish(stats, rstd):
        """Levels 2-3 (DVE), partition-reduce (PE), var math, sqrt (Act) and
        fast reciprocal -> rstd slices for this half."""
        half, sa, qa = stats
        for ci in range(2):
            c = 2 * half + ci
            nc.vector.tensor_tensor(sa[0][ci], sa[0][ci], sa[1][ci], OP.add)
            nc.vector.tensor_tensor(sa[2][ci], sa[2][ci], sa[3][ci], OP.add)
            nc.vector.tensor_tensor(sa[0][ci], sa[0][ci], sa[2][ci], OP.add)
            nc.vector.tensor_tensor(qa[0][ci], qa[0][ci], qa[1][ci], OP.add)
            nc.vector.tensor_tensor(qa[2][ci], qa[2][ci], qa[3][ci], OP.add)
            nc.vector.tensor_tensor(qa[0][ci], qa[0][ci], qa[2][ci], OP.add)
            S = psS.tile([128, 512], F32, name="S", tag="S")
            Q = psS.tile([128, 512], F32, name="Q", tag="Q")
            nc.tensor.matmul(S, ones128, sa[0][ci], start=True, stop=True)
            nc.tensor.matmul(Q, ones128, qa[0][ci], start=True, stop=True)
            mu = statpool.tile([128, 512], BF16, name="mu", tag="mu")
            qd = statpool.tile([128, 512], BF16, name="qd", tag="qd")
            nc.vector.tensor_scalar(mu, S, 1.0 / D, None, OP.mult)
            nc.vector.tensor_scalar(qd, Q, 1.0 / D, None, OP.mult)
            musq = statpool.tile([128, 512], BF16, name="musq", tag="musq")
            nc.vector.tensor_tensor(musq, mu, mu, OP.mult)
            var = statpool.tile([128, 512], BF16, name="var", tag="var")
            nc.vector.tensor_tensor(var, qd, musq, OP.subtract)
            sd = statpool.tile([128, 512], F32, name="sd", tag="sd")
            nc.scalar.activation(sd, var, AF.Sqrt, bias=eps)
            rf = statpool.tile([128, 512], F32, name="rf", tag="rf")
            nc.vector.reciprocal_approx_fast(rf, sd)
            nc.vector.tensor_copy(rstd[:, bass.ts(c, 512)], rf)

    def norm_half(rstd, half):
        """t = y * rstd for this half (mean handled by weight folding)."""
        hs = bass.ts(half, 1024)
        for kt in range(KT):
            nc.vector.tensor_tensor(t_t[:, kt, hs], y_t[:, kt, hs],
                                    rstd[:, hs], OP.mult)

    def mm_half(w_tile, f, half, rhs):
        """8-kt accumulation for fout tile f on this half's two chunks."""
        G = psG.tile([128, 2, 512], F32, name="G", tag="G")
        for kt in range(KT):
            lhsT = w_tile[:, kt, bass.ts(f, 128)]
            for ci in range(2):
                nc.tensor.matmul(G[:, ci, :], lhsT,
                                 rhs[:, kt, bass.ts(2 * half + ci, 512)],
                                 start=(kt == 0), stop=(kt == KT - 1))
        return G

    for _rep in range(repeat):
        # ---------------- Phase 1: y.T = (x @ Win.T + bin).T ----------------
        w_in = wpool.tile([128, KT, D], BF16, tag="w")
        nc.sync.dma_start(w_in, wint.rearrange("(kt p) n -> p kt n", p=128))
        for kt in range(KT):
            nc.sync.dma_start(t_t[:, kt, :],
                              xt.rearrange("(kt p) t -> p kt t", p=128)[:, kt, :])

        rstd_next = rstdpool.tile([128, T], BF16, name="rstd", tag="rstd")
        pending = None  # (stats, rstd, half) whose finish+norm is deferred
        for half in range(NC2):
            st = make_stats(half)
            for f in range(FT):
                G = mm_half(w_in, f, half, t_t)
                nc.scalar.activation(
                    y_t[:, f, bass.ts(half, 1024)],
                    G.rearrange("p a b -> p (a b)"),
                    AF.Identity, bias=bin_t[:, bass.ds(f, 1)])
                if f % 2 == 1:
                    stats_pair(st, f)
                if f == 3 and pending is not None:
                    stats_finish(pending[0], pending[1])
                    norm_half(pending[1], pending[2])
                    pending = None
            pending = (st, rstd_next, half)

        # ---------------- Phase 2: residual blocks ----------------
        for i in range(NB):
            wb = wpool.tile([128, KT, D], BF16, tag="w")
            nc.sync.dma_start(wb, wbt[i].rearrange("(kt p) n -> p kt n", p=128))
            if i < NB - 1:
                rstd_next = rstdpool.tile([128, T], BF16, name="rstd", tag="rstd")
            for half in range(NC2):
                st = make_stats(half) if i < NB - 1 else None
                for f in range(FT):
                    G = mm_half(wb, f, half, t_t)
                    g = gpool.tile([128, 1024], BF16, name="g", tag="g")
                    nc.scalar.activation(
                        g, G.rearrange("p a b -> p (a b)"),
                        AF.Gelu, bias=bb_t[:, i, bass.ds(f, 1)])
                    nc.vector.tensor_tensor(y_t[:, f, bass.ts(half, 1024)],
                                            y_t[:, f, bass.ts(half, 1024)],
                                            g, OP.add)
                    if st is not None and f % 2 == 1:
                        stats_pair(st, f)
                    if f == 3 and pending is not None:
                        stats_finish(pending[0], pending[1])
                        norm_half(pending[1], pending[2])
                        pending = None
                if st is not None:
                    pending = (st, rstd_next, half)

        # ---------------- Phase 3: out.T = (y @ Wout.T + bout).T ------------
        w3a = wpool.tile([128, KT, D], BF16, tag="w")
        nc.sync.dma_start(w3a, woutt[:, 0:D].rearrange("(kt p) n -> p kt n", p=128))
        w3b = wpool.tile([128, KT, D], BF16, tag="w")
        nc.sync.dma_start(w3b, woutt[:, D:NOUT].rearrange("(kt p) n -> p kt n", p=128))
        for half_w, w3 in ((0, w3a), (1, w3b)):
            for f in range(FT):
                fg = half_w * FT + f
                ost = ostpool.tile([128, T], F32, name="ost", tag="ost")
                for half in range(NC2):
                    G = mm_half(w3, f, half, y_t)
                    nc.scalar.activation(
                        ost[:, bass.ts(half, 1024)],
                        G.rearrange("p a b -> p (a b)"),
                        AF.Identity, bias=bout_t[:, bass.ds(fg, 1)])
                nc.gpsimd.dma_start(out_t[bass.ts(fg, 128), :], ost)


_CACHED_NC = None


def _prep_inputs(x, Win, bin_b, Wb, bb, Wout, bout_b):
    import ml_dtypes
    x = np.asarray(x, np.float32)
    Win = np.asarray(Win, np.float32)
    Wb = np.asarray(Wb, np.float32)
    Wout = np.asarray(Wout, np.float32)
    # fold LN mean-subtraction into the inner-block weights:
    # W' = W - outer(rowsum(W), ones)/D  so  W' @ (y*rstd) == ((y-mu)*rstd) @ W.T
    Wbp = Wb - Wb.sum(axis=2, keepdims=True) / D
    xt = np.ascontiguousarray(x.T).astype(ml_dtypes.bfloat16)
    wint = np.ascontiguousarray(Win.T).astype(ml_dtypes.bfloat16)
    wbt = np.ascontiguousarray(Wbp.transpose(0, 2, 1)).astype(ml_dtypes.bfloat16)
    woutt = np.ascontiguousarray(Wout.T).astype(ml_dtypes.bfloat16)
    return (xt, wint, wbt, woutt, np.asarray(bin_b, np.float32),
            np.asarray(bb, np.float32), np.asarray(bout_b, np.float32))


def make_in_maps(x, Win, bin_b, Wb, bb, Wout, bout_b):
    xt, wint, wbt, woutt, bin_arr, bb_arr, bout_arr = _prep_inputs(
        x, Win, bin_b, Wb, bb, Wout, bout_b)
    in_maps = []
    for c in range(N_CORES):
        in_maps.append({
            "xt": np.ascontiguousarray(xt[:, c * T:(c + 1) * T]),
            "wint": wint, "wbt": wbt, "woutt": woutt,
            "bin_b": bin_arr, "bb": bb_arr, "bout": bout_arr,
        })
    return in_maps


def kernel(x, Win, bin_b, Wb, bb, Wout, bout_b):
    global _CACHED_NC
    if _CACHED_NC is None:
        _CACHED_NC = build_nc()
    nc = _CACHED_NC
    in_maps = make_in_maps(x, Win, bin_b, Wb, bb, Wout, bout_b)
    res = bass_utils.run_bass_kernel_spmd(nc, in_maps, list(range(N_CORES)))
    return np.concatenate(
        [np.ascontiguousarray(res.results[c]["out_t"].T) for c in range(N_CORES)],
        axis=0)

